# revision 19
# baseline (speedup 1.0000x reference)
"""Trainium2 Bass kernel for nn_MultiHeadBlock (B=4, S=2048, D=512, H=8).

Sharding: 8 cores = 4 batches x 2 query-halves. Each core computes K/V for its
batch's full 2048-key sequence (duplicated across the pair of cores sharing a
batch; no collectives), and runs all 8 heads for its 1024 queries.

Per-core flow (fp16 matmul operands, fp32 PSUM accumulation):
  x^T shipped pre-transposed fp16 -> QKV projections (pair-0 Q/K first,
  wqkv DMA'd column-sliced so they start ~5us in; the other pairs'
  projections ride inside the attention stream on a dedicated 1-bank PSUM
  tag) -> per-head zero-padded q halves (q_z) let the scores matmul use the
  full 128-row k_t slice as FWL-eligible weights -> head-serial attention:
  scores (PSUM) -> exp on ACT (bf16 out; table preloaded once, no swaps) ->
  V_aug^T @ exp accumulation, software-pipelined two score-groups deep so
  neither PE nor ACT stalls; V_aug head blocks padded to 128-wide weight
  slices (FWL) with a ones column yielding sumexp in the same matmul ->
  per-head 1/sumexp normalization (SBUF-staged: custom DVE ops silently read
  partition 0 for partition-offset PSUM sources) -> O-proj with the residual
  folded into (W_o + I) on the host; qt0's O-proj/LN rides inside qt1's
  attention -> LayerNorm rstd via DVE fast-inverse-sqrt (no ACT Ln table
  swap); output stores alternate the two HWDGE rings.

gamma==1/beta==0 and mask==1 fast paths are chosen at run time from the
actual input values (separately built program variants handle the general
case).
"""

import os
import sys

for _p in ("/opt/trn_rl_repo", "/root/.axon_site/_ro/trn_rl_repo"):
    if os.path.isdir(_p) and _p not in sys.path:
        sys.path.insert(0, _p)

import numpy as np

import concourse.bass as bass
import concourse.bacc as bacc
import concourse.mybir as mybir
import concourse.tile as tile
from concourse.masks import make_identity

F32 = mybir.dt.float32
F16 = mybir.dt.float16
BF16 = mybir.dt.bfloat16
ALU = mybir.AluOpType
ACTF = mybir.ActivationFunctionType

B, S, D = 4, 2048, 512
H, DH = 8, 64
HB = DH + 2          # per-head v_aug block: 64 v cols + ones col + pad col
                     # (66 keeps 2-byte head blocks 4-byte aligned on HW)
VW = H * HB + 64     # v_aug tile width: head 7's 128-wide weight slice needs
                     # cols 462..590; pad to 592 so AV lhsT is always 128 cols
                     # (full-width weights enable FWL so loads hide behind
                     # matmul streaming)
SQ = S // 2          # queries per core
NKT = S // 128       # 16 key row-tiles
NDC = D // 128       # 4 contraction chunks
EPS = 1e-5
N_CORES = 8


def build_program(use_gb=False, use_mask=False, probes=False):
    nc = bacc.Bacc("TRN2", target_bir_lowering=False, debug=False,
                   num_devices=N_CORES)

    xt_d = nc.dram_tensor("xt", [D, S], F16, kind="ExternalInput").ap()
    wqkv_d = nc.dram_tensor("wqkv", [D, 3 * D], F16, kind="ExternalInput").ap()
    bqkv_d = nc.dram_tensor("bqkv_pt", [128, 12], F32, kind="ExternalInput").ap()
    bvrow_d = nc.dram_tensor("bv_row", [1, D], F32, kind="ExternalInput").ap()
    wo_d = nc.dram_tensor("wo", [D, D], F16, kind="ExternalInput").ap()
    borow_d = nc.dram_tensor("bo_row", [1, D], F32, kind="ExternalInput").ap()
    if use_gb:
        gam_d = nc.dram_tensor("gamma_row", [1, D], F32,
                               kind="ExternalInput").ap()
        bet_d = nc.dram_tensor("beta_row", [1, D], F32,
                               kind="ExternalInput").ap()
    if use_mask:
        maskf_d = nc.dram_tensor("maskf_pt", [128, NKT], F32,
                                 kind="ExternalInput").ap()
    out_d = nc.dram_tensor("out", [SQ, D], F32, kind="ExternalOutput").ap()

    KG = [(0, 3), (3, 3), (6, 3), (9, 3), (12, 2), (14, 2)]
    I32 = mybir.dt.int32

    with tile.TileContext(nc) as tc:
        with tc.tile_pool(name="const", bufs=1) as cp, \
             tc.tile_pool(name="a_sb", bufs=2) as asb, \
             tc.tile_pool(name="chunk_sb", bufs=7) as csb, \
             tc.tile_pool(name="ps_sc", bufs=1, space="PSUM") as ps_sc, \
             tc.tile_pool(name="ps_acc", bufs=1, space="PSUM") as ps_acc:

            # ---- input DMAs first; xt on the ACT HWDGE ring, the rest on
            # the SP ring so the two physical rings stream in parallel
            bqkv_sb = cp.tile([128, 12], F32, name="bqkv_sb")
            nc.sync.dma_start(out=bqkv_sb[:], in_=bqkv_d)
            # xt in four 512KB query-major transfers on the ACT ring (one
            # [128, 4, 512] block each = 2KB lines, ~3x the bandwidth of
            # per-dc 1KB-line transfers); wqkv column-sliced on the SP ring,
            # pair-0's Q/K columns first, so the first Q projection starts
            # as soon as the first block lands
            xt_all = cp.tile([128, NDC, S], F16, name="xt_all")
            xt_src = xt_d.rearrange("(dc p) c -> p dc c", p=128)
            for cb in range(4):
                nc.scalar.dma_start(
                    out=xt_all[:, :, cb * 512:(cb + 1) * 512],
                    in_=xt_src[:, :, cb * 512:(cb + 1) * 512])
            xt_sb = [xt_all[:, dc, :] for dc in range(NDC)]
            wq_sb = [cp.tile([128, 3 * D], F16, name=f"wq{dc}")
                     for dc in range(NDC)]

            def wq_dma(lo, hi):
                for dc in range(NDC):
                    nc.sync.dma_start(
                        out=wq_sb[dc][:, lo:hi],
                        in_=wqkv_d[dc * 128:(dc + 1) * 128, lo:hi])

            wq_dma(0, 128)          # Q cols, pair 0
            wq_dma(D, D + 128)      # K cols, pair 0
            nrow = 4 if use_gb else 2
            rows = cp.tile([1, nrow * D], F32, name="rows")
            nc.sync.dma_start(out=rows[0:1, 0:D], in_=bvrow_d)
            nc.sync.dma_start(out=rows[0:1, D:2 * D], in_=borow_d)
            if use_gb:
                nc.sync.dma_start(out=rows[0:1, 2 * D:3 * D], in_=gam_d)
                nc.sync.dma_start(out=rows[0:1, 3 * D:4 * D], in_=bet_d)
            wq_dma(2 * D, 3 * D)    # V cols
            wq_dma(128, D)          # Q cols, pairs 1-3
            wq_dma(D + 128, 2 * D)  # K cols, pairs 1-3
            wo_sb = []
            for c in range(NDC):
                w = cp.tile([128, D], F16, name=f"wo{c}")
                nc.sync.dma_start(out=w[:], in_=wo_d[c * 128:(c + 1) * 128, :])
                wo_sb.append(w)
            if use_mask:
                maskf_sb = cp.tile([128, NKT], F32, name="maskf_sb")
                nc.sync.dma_start(out=maskf_sb[:], in_=maskf_d)

            ones8 = cp.tile([128, 8], F32, name="ones8")
            nc.vector.memset(ones8[:], 1.0)
            # preload the exp ACT table before the attention stream begins
            actwarm = cp.tile([1, 1], F32, name="actwarm")
            nc.scalar.activation(actwarm[0:1, 0:1], ones8[0:1, 0:1], ACTF.Exp)

            ident_f = cp.tile([128, 128], F32, name="ident_f")
            make_identity(nc, ident_f[:])
            ident = cp.tile([128, 128], F16, name="ident")
            nc.vector.tensor_copy(ident[:], ident_f[:])


            q_t = [cp.tile([128, SQ], F16, name=f"qt{t}") for t in range(4)]
            # per-head q with the other head's partitions zeroed: the scores
            # matmul can then use the full 128-row k_t slice as weights (one
            # FWL-eligible load shared by both heads)
            q_z = [[cp.tile([128, SQ], F16, name=f"qz{t}_{e}")
                    for e in range(2)] for t in range(4)]
            k_t = [cp.tile([128, S], F16, name=f"kt{t}") for t in range(4)]
            v_aug = [cp.tile([128, VW], BF16, name=f"va{t}")
                     for t in range(NKT)]
            x_sb = [cp.tile([128, D], F32, name=f"x{i}") for i in range(8)]
            sumx8 = cp.tile([128, 8], F32, name="sumx8")
            sumsq8 = cp.tile([128, 8], F32, name="sumsq8")

            # pair-0's q_z zero halves must precede its first scores; all
            # other init DVE work is deferred past the pair-0 projections so
            # it doesn't delay the first score group
            nc.vector.memset(q_z[0][0][64:128, :], 0.0)
            nc.vector.memset(q_z[0][1][0:64, :], 0.0)

            def late_inits():
                for t in range(1, 4):
                    nc.vector.memset(q_z[t][0][64:128, :], 0.0)
                    nc.vector.memset(q_z[t][1][0:64, :], 0.0)
                for rt in range(NKT):
                    va_pad = v_aug[rt][:, 0:H * HB].rearrange(
                        "p (h c) -> p h c", c=HB)[:, :, DH + 1:DH + 2]
                    nc.vector.memset(va_pad, 0.0)
                    nc.vector.memset(v_aug[rt][:, H * HB:VW], 0.0)
                if not use_mask:
                    on_v = ones8[:, :].rearrange("p (h c) -> p h c", c=1)
                    for rt in range(NKT):
                        va_one = v_aug[rt][:, 0:H * HB].rearrange(
                            "p (h c) -> p h c", c=HB)[:, :, DH:DH + 1]
                        nc.vector.tensor_copy(va_one, on_v)
                eps_col = cp.tile([128, 1], F32, name="eps_col")
                nc.vector.memset(eps_col[:], EPS)
                cst1 = cp.tile([128, 4], I32, name="cst1")
                nc.vector.memset(cst1[:], 1)
                cstnot = cp.tile([128, 4], I32, name="cstnot")
                nc.vector.memset(cstnot[:], -1)
                cstmag = cp.tile([128, 4], I32, name="cstmag")
                nc.vector.memset(cstmag[:], 0x5f3759e0)
                return eps_col, cst1, cstnot, cstmag

            # PE warmup: ~4us of dummy matmuls so HAM unthrottles before the
            # first real matmul (input DMAs overlap this)
            wps = ps_acc.tile([128, 512], F32, tag="qkv", name="wps")
            for _ in range(36):
                nc.tensor.matmul(wps[0:128, 0:128], lhsT=ident[:],
                                 rhs=ident[:], start=True, stop=True)

            bv_bc = cp.tile([128, D], F32, name="bv_bc")
            bo_bc = cp.tile([128, D], F32, name="bo_bc")
            bcs = [bv_bc, bo_bc]
            if use_gb:
                gam_bc = cp.tile([128, D], F32, name="gam_bc")
                bet_bc = cp.tile([128, D], F32, name="bet_bc")
                bcs += [gam_bc, bet_bc]
            for j, t in enumerate(bcs):
                nc.gpsimd.partition_broadcast(
                    t[:], rows[0:1, j * D:(j + 1) * D], channels=128)

            # ---- QKV projection pieces. During attention these are issued
            # one piece per score-group, using a dedicated 1-bank PSUM tag so
            # they overlap the ACT-bound attention stream instead of
            # squeezing between pairs.
            def q_piece(t4, qh, tag="qkv"):
                ps = ps_acc.tile([128, 512], F32, tag=tag,
                                 name=f"qps{t4}_{qh}")
                for dc in range(NDC):
                    nc.tensor.matmul(
                        ps[:],
                        lhsT=wq_sb[dc][:, t4 * 128:(t4 + 1) * 128],
                        rhs=xt_sb[dc][:, qh * 512:(qh + 1) * 512],
                        start=(dc == 0), stop=(dc == NDC - 1))
                nc.vector.tensor_scalar_add(
                    out=q_t[t4][:, qh * 512:(qh + 1) * 512],
                    in0=ps[:], scalar1=bqkv_sb[:, t4:t4 + 1])
                if qh == SQ // 512 - 1:
                    nc.vector.tensor_copy(q_z[t4][0][0:64, :],
                                          q_t[t4][0:64, :])
                    nc.vector.tensor_copy(q_z[t4][1][64:128, :],
                                          q_t[t4][64:128, :])

            def k_piece(t4, kq, tag="qkv"):
                ps = ps_acc.tile([128, 512], F32, tag=tag,
                                 name=f"kps{t4}_{kq}")
                for dc in range(NDC):
                    nc.tensor.matmul(
                        ps[:],
                        lhsT=wq_sb[dc][:, D + t4 * 128:D + (t4 + 1) * 128],
                        rhs=xt_sb[dc][:, kq * 512:(kq + 1) * 512],
                        start=(dc == 0), stop=(dc == NDC - 1))
                nc.vector.tensor_scalar_add(
                    out=k_t[t4][:, kq * 512:(kq + 1) * 512],
                    in0=ps[:], scalar1=bqkv_sb[:, 4 + t4:5 + t4])

            def v_piece(rt, tag="qkv"):
                ps = ps_acc.tile([128, 512], F32, tag=tag, name=f"vps{rt}")
                for dc in range(NDC):
                    nc.tensor.matmul(
                        ps[:],
                        lhsT=xt_sb[dc][:, rt * 128:(rt + 1) * 128],
                        rhs=wq_sb[dc][:, 2 * D:3 * D],
                        start=(dc == 0), stop=(dc == NDC - 1))
                va_v = v_aug[rt][:, 0:H * HB].rearrange(
                    "p (h c) -> p h c", c=HB)[:, :, 0:DH]
                if use_mask:
                    vtmp = asb.tile([128, 512], F32, tag="vtmp",
                                    name=f"vtmp{rt}")
                    nc.vector.tensor_add(vtmp[:], ps[:], bv_bc[:])
                    vt_v = vtmp[:, :].rearrange("p (h c) -> p h c", c=DH)
                    nc.vector.tensor_scalar_mul(
                        out=va_v, in0=vt_v, scalar1=maskf_sb[:, rt:rt + 1])
                    va_one = v_aug[rt][:, 0:H * HB].rearrange(
                        "p (h c) -> p h c", c=HB)[:, :, DH:DH + 1]
                    on_v = ones8[:, :].rearrange("p (h c) -> p h c", c=1)
                    nc.vector.tensor_scalar_mul(
                        out=va_one, in0=on_v, scalar1=maskf_sb[:, rt:rt + 1])
                else:
                    ps_v = ps[:, :].rearrange("p (h c) -> p h c", c=DH)
                    bv_v = bv_bc[:, :].rearrange("p (h c) -> p h c", c=DH)
                    nc.vector.tensor_add(va_v, ps_v, bv_v)

            _carry = [None]

            def flush_carry():
                if _carry[0] is not None:
                    pends, av_fn, nfn = _carry[0]
                    _carry[0] = None
                    for p in pends:
                        av_fn(p)
                    nfn()

            def score_head(qt, pair, e, nbufs=6, at_tag="at"):
                # scores+exp only (no AV): used to start the scalar engine
                # before the V projection exists
                pends = []
                for k0, kl in KG:
                    sc = ps_sc.tile([128, kl * 512], F32, tag="sc", bufs=2,
                                    name=f"psc{qt}_{pair}_{e}_{k0}")
                    for j in range(kl):
                        kc = k0 + j
                        nc.tensor.matmul(
                            sc[:, j * 512:(j + 1) * 512],
                            lhsT=k_t[pair][:, kc * 128:(kc + 1) * 128],
                            rhs=q_z[pair][e][:, qt * 512:(qt + 1) * 512],
                            start=True, stop=True)
                    at = asb.tile([128, kl * 512], BF16, tag=at_tag,
                                  bufs=nbufs, name=f"pat{qt}_{pair}_{e}_{k0}")
                    nc.scalar.activation(at[:], sc[:], ACTF.Exp)
                    pends.append((at, k0, kl))
                return pends

            def attention(qt, pair, units, pre0=None):
                # head-serial with cross-boundary carry: each head's trailing
                # AV flush + normalization is deferred until after the NEXT
                # head-stream's first score group has been issued, so the
                # scalar engine never waits through a boundary drain.
                chunk = csb.tile([128, 512], F16, tag="chunk",
                                 name=f"chunk{qt}_{pair}")
                for e in range(2):
                    acc = ps_acc.tile([128, 512], F32, tag="acc",
                                      name=f"acc{qt}_{pair}_{e}")
                    h = 2 * pair + e

                    def issue_av(p, acc=acc, h=h):
                        p_at, p_k0, p_kl = p
                        for j in range(p_kl):
                            kc = p_k0 + j
                            nc.tensor.matmul(
                                acc[:],
                                lhsT=v_aug[kc][:, h * HB:h * HB + 128],
                                rhs=p_at[:, j * 512:(j + 1) * 512],
                                start=(kc == 0), stop=(kc == NKT - 1))

                    def norm_fn(acc=acc, e=e, chunk=chunk, qt=qt, pair=pair):
                        # per-head 1/sumexp normalization. The au copy frees
                        # the single-buffer PSUM acc tag immediately so the
                        # next head's AV accumulation is not blocked through
                        # the reciprocal/broadcast chain; custom DVE ops also
                        # need their source staged at partition 0 (they
                        # silently read partition 0 for partition-offset PSUM
                        # sources).
                        au = asb.tile([DH + 1, 512], F32, tag="au", bufs=2,
                                      name=f"au{qt}_{pair}_{e}")
                        nc.vector.tensor_copy(au[:], acc[0:DH + 1, :])
                        se0 = asb.tile([1, 512], F32, tag="se0",
                                       name=f"se{qt}_{pair}_{e}")
                        nc.vector.tensor_copy(se0[0:1, :], au[DH:DH + 1, :])
                        rs1 = asb.tile([1, 512], F32, tag="rs1",
                                       name=f"rs{qt}_{pair}_{e}")
                        nc.vector.reciprocal_approx_fast(
                            out=rs1[0:1, :], in_=se0[0:1, :])
                        rp = asb.tile([64, 512], F32, tag="rp",
                                      name=f"rp{qt}_{pair}_{e}")
                        nc.gpsimd.partition_broadcast(
                            rp[:], rs1[0:1, :], channels=64)
                        nc.vector.tensor_mul(
                            chunk[64 * e:64 * (e + 1), :], au[0:DH, :],
                            rp[:])

                    if e == 0 and pre0 is not None:
                        # pop the previous carry, then defer this prescored
                        # head's own flush into e1's first score group so ACT
                        # restarts immediately
                        flush_carry()
                        _carry[0] = (pre0, issue_av, norm_fn)
                    else:
                        # depth-2 pipeline: the AV for group g issues after
                        # the scores of group g+2, so exp(g) has two groups
                        # of PE work to hide behind
                        pends = []
                        first = True
                        for k0, kl in KG:
                            sc = ps_sc.tile(
                                [128, kl * 512], F32, tag="sc", bufs=2,
                                name=f"sc{qt}_{pair}_{e}_{k0}")
                            for j in range(kl):
                                kc = k0 + j
                                nc.tensor.matmul(
                                    sc[:, j * 512:(j + 1) * 512],
                                    lhsT=k_t[pair][:,
                                             kc * 128:(kc + 1) * 128],
                                    rhs=q_z[pair][e][:,
                                            qt * 512:(qt + 1) * 512],
                                    start=True, stop=True)
                            if first:
                                flush_carry()
                                first = False
                            if len(pends) == 2:
                                issue_av(pends.pop(0))
                            at = asb.tile([128, kl * 512], BF16, tag="at",
                                          bufs=6,
                                          name=f"at{qt}_{pair}_{e}_{k0}")
                            nc.scalar.activation(at[:], sc[:], ACTF.Exp)
                            if units:
                                units.pop(0)()
                            pends.append((at, k0, kl))
                        _carry[0] = (pends, issue_av, norm_fn)
                while units:
                    units.pop(0)()
                return chunk

            def oproj_piece(qt, chunks, qsub, tag, tail=False):
                i = qt * 4 + qsub
                po = ps_acc.tile([128, 512], F32, tag=tag, name=f"po{i}")
                for c in range(NDC):
                    nc.tensor.matmul(
                        po[:],
                        lhsT=chunks[c][:, qsub * 128:(qsub + 1) * 128],
                        rhs=wo_sb[c][:],
                        start=(c == 0), stop=(c == NDC - 1))
                nc.vector.scalar_tensor_tensor(
                    out=x_sb[i][:], in0=po[:], scalar=0.0,
                    in1=bo_bc[:], op0=ALU.add, op1=ALU.add,
                    accum_out=sumx8[:, i:i + 1])
                sq = asb.tile([128, 512], F32, tag="sq", name=f"sq{i}")
                if tail:
                    # ACT is idle in the tail; square lives in every table
                    nc.scalar.activation(sq[:], x_sb[i][:], ACTF.Square,
                                         accum_out=sumsq8[:, i:i + 1])
                else:
                    nc.vector.scalar_tensor_tensor(
                        out=sq[:], in0=x_sb[i][:], scalar=0.0,
                        in1=x_sb[i][:], op0=ALU.add, op1=ALU.mult,
                        accum_out=sumsq8[:, i:i + 1])

            def finish_qt(qt, chunks, skip_oproj=False, tail=False):
                if tail:
                    # prefetch the sqrt table with a dummy op so the swap
                    # overlaps the tail norm instead of the stats chain
                    # (square/identity live in every table set)
                    nc.scalar.activation(actwarm[0:1, 0:1], ones8[0:1, 0:1],
                                         ACTF.Sqrt)
                if not skip_oproj:
                    for qsub in range(4):
                        oproj_piece(qt, chunks, qsub,
                                    "acc" if qsub % 2 else "qkv", tail=tail)

                c0 = qt * 4
                mu4 = asb.tile([128, 4], F32, tag="mu4", name=f"mu4_{qt}")
                nc.vector.tensor_scalar_mul(
                    out=mu4[:], in0=sumx8[:, c0:c0 + 4], scalar1=1.0 / D)
                var4 = asb.tile([128, 4], F32, tag="var4", name=f"var4_{qt}")
                nc.vector.tensor_scalar_mul(
                    out=var4[:], in0=sumsq8[:, c0:c0 + 4], scalar1=1.0 / D)
                msq = asb.tile([128, 4], F32, tag="msq", name=f"msq{qt}")
                nc.vector.scalar_tensor_tensor(
                    out=msq[:], in0=mu4[:], scalar=-1.0,
                    in1=mu4[:], op0=ALU.mult, op1=ALU.mult)
                nc.vector.tensor_add(var4[:], var4[:], msq[:])
                rstd4 = asb.tile([128, 4], F32, tag="rstd4",
                                 name=f"rstd4_{qt}")
                if tail:
                    # std = Sqrt(var + eps) on the idle ACT engine (table
                    # prefetched above), then a fast DVE reciprocal
                    std4 = asb.tile([128, 4], F32, tag="std4",
                                    name=f"sd{qt}")
                    nc.scalar.activation(std4[:], var4[:], ACTF.Sqrt,
                                         bias=eps_col[:, 0:1])
                    nc.vector.reciprocal_approx_fast(out=rstd4[:],
                                                     in_=std4[:])
                else:
                    nc.vector.tensor_scalar_add(out=var4[:], in0=var4[:],
                                                scalar1=EPS)
                    # rstd = 1/sqrt(var) on DVE (fast-inverse-sqrt seed +
                    # two Newton steps, ~2e-6 relative) - no table swap
                    sh = asb.tile([128, 4], I32, tag="sh", name=f"sh{qt}")
                    nc.vector.tensor_tensor(sh[:], var4[:].bitcast(I32),
                                            cst1[:],
                                            ALU.logical_shift_right)
                    nc.vector.tensor_tensor(sh[:], sh[:], cstnot[:],
                                            ALU.bitwise_xor)
                    nc.vector.tensor_tensor(sh[:], sh[:], cstmag[:], ALU.add)
                    y0 = sh[:].bitcast(F32)
                    tmp4 = asb.tile([128, 4], F32, tag="tmp4",
                                    name=f"tmp4_{qt}")
                    for it in range(2):
                        yin = y0 if it == 0 else rstd4[:]
                        nc.vector.tensor_mul(tmp4[:], yin, yin)
                        nc.vector.tensor_mul(tmp4[:], tmp4[:], var4[:])
                        nc.vector.tensor_scalar(
                            out=tmp4[:], in0=tmp4[:], scalar1=-0.5,
                            scalar2=1.5, op0=ALU.mult, op1=ALU.add)
                        nc.vector.tensor_mul(rstd4[:], yin, tmp4[:])
                if tail:
                    nbias = asb.tile([128, 4], F32, tag="nbias",
                                     name=f"nb{qt}")
                    nc.vector.scalar_tensor_tensor(
                        out=nbias[:], in0=mu4[:], scalar=-1.0,
                        in1=rstd4[:], op0=ALU.mult, op1=ALU.mult)
                for qsub in range(4):
                    i = qt * 4 + qsub
                    y = x_sb[i]  # in place: no store-gated buffer stalls
                    if tail and qsub % 2 == 0:
                        nc.scalar.activation(
                            y[:], x_sb[i][:], ACTF.Identity,
                            bias=nbias[:, qsub:qsub + 1],
                            scale=rstd4[:, qsub:qsub + 1])
                    else:
                        nc.vector.tensor_scalar(
                            out=y[:], in0=x_sb[i][:],
                            scalar1=mu4[:, qsub:qsub + 1],
                            scalar2=rstd4[:, qsub:qsub + 1],
                            op0=ALU.subtract, op1=ALU.mult)
                    if use_gb:
                        nc.vector.tensor_mul(y[:], y[:], gam_bc[:])
                        nc.vector.tensor_add(y[:], y[:], bet_bc[:])
                    # alternate HWDGE rings so the final stores overlap
                    eng = nc.sync if qsub % 2 == 0 else nc.scalar
                    eng.dma_start(out=out_d[i * 128:(i + 1) * 128, :],
                                  in_=y[:])

            # ---- schedule: Q/K for pair 0 and all of V upfront (PSUM tags
            # alternate so the projections pipeline), then attention with the
            # remaining pairs' Q/K slotted into the stream
            tags = ["qkv", "acc"]
            n = 0
            for qh in range(SQ // 512):
                q_piece(0, qh, tags[n % 2]); n += 1
            for kq in range(S // 512):
                k_piece(0, kq, tags[n % 2]); n += 1
            eps_col, cst1, cstnot, cstmag = late_inits()
            # pair0/head0's scores+exps only need Q/K: issue them before the
            # V projections so the scalar engine starts ~14us earlier
            pre0 = score_head(0, 0, 0)
            # second prescore: qt1/pair0/head0 also only needs k_t[0]/q_z[0];
            # its exps keep ACT busy through the V projections, and its at
            # tiles park in dedicated buffers until attention(1,0)
            pre1 = score_head(1, 0, 0, at_tag="atp")
            for rt in range(NKT):
                v_piece(rt, tags[n % 2]); n += 1

            chunks0 = []
            for pair in range(4):
                units = []
                if pair < 3:
                    t4 = pair + 1
                    units = [
                        (lambda t4=t4, qh=qh: q_piece(t4, qh))
                        for qh in range(SQ // 512)
                    ] + [
                        (lambda t4=t4, kq=kq: k_piece(t4, kq))
                        for kq in range(S // 512)
                    ]
                chunks0.append(attention(0, pair, units,
                                         pre0 if pair == 0 else None))
            chunks1 = []
            for pair in range(4):
                units = []
                if pair == 1:
                    # qt0's O-proj/LN rides inside qt1's attention stream so
                    # the qt boundary has no ACT gap
                    units = [
                        (lambda qsub=qsub: oproj_piece(0, chunks0, qsub,
                                                       "qkv"))
                        for qsub in range(4)
                    ] + [lambda: finish_qt(0, chunks0, skip_oproj=True)]
                chunks1.append(attention(1, pair, units,
                                         pre1 if pair == 0 else None))
            flush_carry()
            finish_qt(1, chunks1, tail=True)
    nc.compile()
    return nc


_CACHED = {}


def _variant(mask=None, gamma=None, beta=None):
    use_gb = not (gamma is None or
                  (np.allclose(gamma, 1.0) and np.allclose(beta, 0.0)))
    use_mask = not (mask is None or bool(np.all(np.asarray(mask) > 0)))
    return use_gb, use_mask


def _get_program(use_gb=False, use_mask=False):
    key = (use_gb, use_mask)
    if key not in _CACHED:
        _CACHED[key] = build_program(use_gb=use_gb, use_mask=use_mask)
    return _CACHED[key]


def make_in_maps(inputs, mask, W_qkv, b_qkv, W_o, b_o, gamma, beta):
    inputs = np.asarray(inputs, np.float32)
    mask = np.asarray(mask)
    W_qkv = np.asarray(W_qkv, np.float32)
    b_qkv = np.asarray(b_qkv, np.float32)
    W_o = np.asarray(W_o, np.float32)
    b_o = np.asarray(b_o, np.float32)
    gamma = np.asarray(gamma, np.float32)
    beta = np.asarray(beta, np.float32)
    use_gb, use_mask = _variant(mask, gamma, beta)

    wo_res = W_o + np.eye(D, dtype=np.float32)   # residual folded in
    shared = {
        "wqkv": np.ascontiguousarray(W_qkv.astype(np.float16)),
        "bqkv_pt": np.ascontiguousarray(b_qkv.reshape(12, 128).T),
        "bv_row": np.ascontiguousarray(b_qkv[2 * D:3 * D].reshape(1, D)),
        "wo": np.ascontiguousarray(wo_res.astype(np.float16)),
        "bo_row": np.ascontiguousarray(b_o.reshape(1, D)),
    }
    if use_gb:
        shared["gamma_row"] = np.ascontiguousarray(gamma.reshape(1, D))
        shared["beta_row"] = np.ascontiguousarray(beta.reshape(1, D))
    in_maps = []
    for c in range(N_CORES):
        b, half = divmod(c, 2)
        xb = inputs[b]
        mk = mask[b].astype(np.float32)
        if half:
            order = np.r_[SQ:S, 0:SQ]
            xb = xb[order]
            mk = mk[order]
        m = dict(shared)
        m["xt"] = np.ascontiguousarray(xb.T.astype(np.float16))
        if use_mask:
            m["maskf_pt"] = np.ascontiguousarray(mk.reshape(NKT, 128).T)
        in_maps.append(m)
    return in_maps


def kernel(inputs, mask, W_qkv, b_qkv, W_o, b_o, gamma, beta):
    from concourse.bass_utils import run_bass_kernel_spmd

    use_gb, use_mask = _variant(mask, np.asarray(gamma), np.asarray(beta))
    nc = _get_program(use_gb, use_mask)
    in_maps = make_in_maps(inputs, mask, W_qkv, b_qkv, W_o, b_o, gamma, beta)
    res = run_bass_kernel_spmd(nc, in_maps, list(range(N_CORES)))
    out = np.empty((B, S, D), np.float32)
    for c in range(N_CORES):
        b, half = divmod(c, 2)
        out[b, half * SQ:(half + 1) * SQ, :] = res.results[c]["out"]
    return out

